# revision 2
# baseline (speedup 1.0000x reference)
"""RBF-softmax grid pooling (CViT) on 8 TRN2 NeuronCores.

out[b,n,c] = sum_g softmax_g(-EPS*|coords_n - grid_g|^2) * x[b,g,c]

EPS=1e5 makes the softmax numerically supported on the 2x2x2 grid box around
each query: every non-box weight is < 1e-11 relative (below f32 resolution of
the sum), so the kernel gathers the 8 box corners per query and reproduces the
reference computation exactly on them, including XLA-CPU f32 exp flush-to-zero
semantics (exp(a)==0 iff a < -87.33654f) and the resulting 0/0 -> NaN rows.

Sharding: queries split across 8 cores (512 each); x replicated (gather table).
"""
import numpy as np
import concourse.bass as bass
import concourse.mybir as mybir
from concourse.bass_utils import run_bass_kernel_spmd
from contextlib import ExitStack

F32 = mybir.dt.float32
I32 = mybir.dt.int32
AF = mybir.ActivationFunctionType
OP = mybir.AluOpType

B, H, W, D, C = 2, 64, 64, 16, 128
N = 4096
NCORES = 8
NS = N // NCORES            # 512 queries per core
NCH = NS // 128             # 4 chunks of 128 queries
EPS = 100000.0
T_CUT = float(np.float32(-87.33654))    # last f32 input with XLA exp > 0
F_ZERO = float(np.float32(-87.33655))   # first f32 input with XLA exp == 0
ROW_B = H * W * D                       # rows per batch in (B*H*W*D, C) table

_NC_CACHE = {}


def _build():
    nc = bass.Bass()
    x_p = nc.declare_dram_parameter("x", [B, H, W, D, C], F32, isOutput=False)
    co_p = nc.declare_dram_parameter("coords", [NS, 3], F32, isOutput=False)
    cs_p = nc.declare_dram_parameter("cst", [128, 37], F32, isOutput=False)
    out_p = nc.declare_dram_parameter("out", [B, NS, C], F32, isOutput=True)

    x_tab = x_p[:].rearrange("b h w d c -> (b h w d) c")

    with ExitStack() as ctx:
        def sb(name, shape, dt=F32):
            return ctx.enter_context(nc.sbuf_tensor(name, shape, dt))

        Cc = sb("Cc", [128, 12])        # coords: col = ch*3 + ax
        csb = sb("csb", [128, 37])      # scl(12) kcol(12) mul12(12) nan(1)
        Tt = sb("Tt", [128, 12])
        TM = sb("TM", [128, 12])
        I0i = sb("I0i", [128, 12], I32)
        I0f = sb("I0f", [128, 12])
        G0 = sb("G0", [128, 12])
        G1p = sb("G1p", [128, 12])
        G1 = sb("G1", [128, 12])
        RBF = sb("RBF", [128, 12])
        RBS = sb("RBS", [128, 4])
        DT = sb("DT", [128, 24])        # col = r*12 + ch*3 + ax
        QT = sb("QT", [128, 24])
        IDX = sb("IDX", [128, 4], I32)
        D2 = sb("D2", [128, 32])        # col = ch*8 + c, c = r*4+s*2+z
        Aa = sb("Aa", [128, 32])
        AM = sb("AM", [128, 32])
        MK = sb("MK", [128, 32])
        Ee = sb("Ee", [128, 32])
        EM = sb("EM", [128, 32])
        DEN = sb("DEN", [128, 4])
        RD = sb("RD", [128, 4])
        ZC = sb("ZC", [128, 4])
        IZ = sb("IZ", [128, 4])
        IZI = sb("IZI", [128, 4], I32)
        NB = sb("NB", [128, 4], I32)
        Ww = sb("Ww", [128, 32])
        F_t = [sb(f"F{g}", [128, 256]) for g in range(32)]       # gathered rows
        P_t = [sb(f"P{k}", [128, 128]) for k in range(64)]       # weighted halves
        LA = [[sb(f"LA{p}_{j}", [128, 128]) for j in range(4)] for p in range(2)]
        LB = [[sb(f"LB{p}_{j}", [128, 128]) for j in range(2)] for p in range(2)]
        ACC = [sb(f"ACC{bi}", [128, 128]) for bi in range(8)]

        sd = ctx.enter_context(nc.semaphore("sd"))
        sv = ctx.enter_context(nc.semaphore("sv"))
        sa = ctx.enter_context(nc.semaphore("sa"))
        sg = ctx.enter_context(nc.semaphore("sg"))
        ss = ctx.enter_context(nc.semaphore("ss"))
        so = ctx.enter_context(nc.semaphore("so"))
        block = ctx.enter_context(nc.Block())

        scl = csb[:, 0:12]
        kcol = csb[:, 12:24]
        mul12 = csb[:, 24:36]
        nanb = csb[:, 36:37]

        @block.sync
        def _(sync):
            sync.dma_start(
                Cc[:].rearrange("p (ch ax) -> p ch ax", ch=NCH, ax=3),
                co_p[:].rearrange("(ch p) ax -> p ch ax", p=128),
            ).then_inc(sd, 16)
            sync.dma_start(csb[:], cs_p[:]).then_inc(sd, 16)
            for bi in range(8):
                ch, b = bi // 2, bi % 2
                sync.wait_ge(ss, bi + 1)
                sync.dma_start(out_p[b, ch * 128:(ch + 1) * 128, :],
                               ACC[bi][:]).then_inc(so, 16)
            sync.wait_ge(so, 128)

        @block.gpsimd
        def _(gpsimd):
            gpsimd.wait_ge(sv, 1)       # IDX ready
            for ch in range(NCH):
                for b in range(B):
                    for rs in range(4):
                        r, s = rs >> 1, rs & 1
                        eo = (b * ROW_B + r * W * D + s * D) * C
                        gpsimd.indirect_dma_start(
                            out=F_t[ch * 8 + b * 4 + rs][:],
                            out_offset=None,
                            in_=x_tab,
                            in_offset=bass.IndirectOffsetOnAxis(
                                ap=IDX[:, ch:ch + 1], axis=0),
                            element_offset=eo,
                        ).then_inc(sg, 16)

        @block.scalar
        def _(scalar):
            scalar.wait_ge(sv, 2)       # AM ready
            scalar.activation(Ee[:], AM[:], AF.Exp).then_inc(sa, 1)
            scalar.wait_ge(sv, 3)       # W ready
            for bi in range(8):
                ch = bi // 2
                scalar.wait_ge(sg, 16 * (4 * bi + 4))
                for rs in range(4):
                    for z in range(2):
                        scalar.activation(
                            P_t[bi * 8 + rs * 2 + z][:],
                            F_t[4 * bi + rs][:, z * 128:(z + 1) * 128],
                            AF.Copy,
                            scale=Ww[:, ch * 8 + rs * 2 + z: ch * 8 + rs * 2 + z + 1],
                        ).then_inc(sa, 1)

        @block.vector
        def _(v):
            v.wait_ge(sd, 32)
            v.tensor_tensor(Tt[:], Cc[:], scl, OP.mult)
            v.drain()
            v.tensor_scalar_sub(TM[:], Tt[:], 0.5)
            v.drain()
            v.tensor_copy(I0i[:], TM[:])            # rint(t-0.5) == floor(t)
            v.drain()
            v.tensor_copy(I0f[:], I0i[:])
            v.drain()
            v.tensor_tensor(G0[:], I0f[:], kcol, OP.mult)
            v.tensor_tensor(RBF[:], I0f[:], mul12, OP.mult)
            v.tensor_scalar_add(G1p[:], I0f[:], 1.0)
            v.drain()
            v.tensor_tensor(G1[:], G1p[:], kcol, OP.mult)
            v.tensor_reduce(RBS[:], RBF[:].rearrange("p (ch ax) -> p ch ax", ch=NCH),
                            axis=mybir.AxisListType.X, op=OP.add)
            v.drain()
            v.tensor_tensor(DT[:, 0:12], Cc[:], G0[:], OP.subtract)
            v.tensor_tensor(DT[:, 12:24], Cc[:], G1[:], OP.subtract)
            v.tensor_copy(IDX[:], RBS[:])
            v.drain()
            v.tensor_tensor(QT[:], DT[:], DT[:], OP.mult).then_inc(sv, 1)  # IDX ready
            v.drain()
            # D2[:, ch*8 + c] = (QX_r + QY_s) + QZ_z
            QTv = QT[:].rearrange("p (r ch ax) -> p r ch ax", r=2, ch=NCH, ax=3)
            D2v = D2[:].rearrange("p (ch c) -> p c ch", ch=NCH, c=8)
            for c in range(8):
                r, s, z = (c >> 2) & 1, (c >> 1) & 1, c & 1
                v.tensor_tensor(D2v[:, c], QTv[:, r, :, 0], QTv[:, s, :, 1], OP.add)
            v.drain()
            for c in range(8):
                z = c & 1
                v.tensor_tensor(D2v[:, c], D2v[:, c], QTv[:, z, :, 2], OP.add)
            v.drain()
            v.tensor_scalar_mul(Aa[:], D2[:], -EPS)
            v.drain()
            v.tensor_scalar_max(AM[:], Aa[:], T_CUT)
            v.tensor_scalar(MK[:], Aa[:], F_ZERO, 1e9, OP.subtract, OP.mult)
            v.drain()
            v.tensor_scalar(MK[:], MK[:], 0.0, 1.0, OP.max, OP.min).then_inc(sv, 1)
            v.drain()
            v.wait_ge(sa, 1)            # E ready
            v.tensor_tensor(EM[:], Ee[:], MK[:], OP.mult)
            v.drain()
            v.tensor_reduce(DEN[:], EM[:].rearrange("p (ch c) -> p ch c", ch=NCH),
                            axis=mybir.AxisListType.X, op=OP.add)
            v.drain()
            v.reciprocal(RD[:], DEN[:])
            v.tensor_scalar(ZC[:], DEN[:], 3.4e38, 1.0, OP.mult, OP.min)
            v.drain()
            v.tensor_scalar_sub(IZ[:], ZC[:], 1.0)      # -1 dead, 0 alive
            v.drain()
            v.tensor_copy(IZI[:], IZ[:])
            v.drain()
            v.tensor_tensor(NB[:], IZI[:],
                            nanb.bitcast(I32).to_broadcast([128, 4]),
                            OP.bitwise_and)
            for ch in range(NCH):
                v.tensor_scalar_mul(Ww[:, ch * 8:(ch + 1) * 8],
                                    EM[:, ch * 8:(ch + 1) * 8],
                                    RD[:, ch:ch + 1])
            v.drain()
            last = None
            for ch in range(NCH):
                last = v.tensor_tensor(
                    Ww[:, ch * 8:(ch + 1) * 8].bitcast(I32),
                    Ww[:, ch * 8:(ch + 1) * 8].bitcast(I32),
                    NB[:, ch:ch + 1].to_broadcast([128, 8]),
                    OP.bitwise_or)
            last.then_inc(sv, 1)        # W ready
            v.drain()
            for bi in range(8):
                par = bi % 2
                v.wait_ge(sa, 1 + 8 * (bi + 1))
                for j in range(4):
                    v.tensor_tensor(LA[par][j][:], P_t[bi * 8 + 2 * j][:],
                                    P_t[bi * 8 + 2 * j + 1][:], OP.add)
                v.drain()
                v.tensor_tensor(LB[par][0][:], LA[par][0][:], LA[par][1][:], OP.add)
                v.tensor_tensor(LB[par][1][:], LA[par][2][:], LA[par][3][:], OP.add)
                v.drain()
                v.tensor_tensor(ACC[bi][:], LB[par][0][:], LB[par][1][:],
                                OP.add).then_inc(ss, 1)
    return nc


def _consts():
    cst = np.zeros((128, 37), dtype=np.float32)
    scl = np.array([H - 1, W - 1, D - 1] * NCH, dtype=np.float32)
    kcol = np.array([np.float32(1.0) / np.float32(H - 1),
                     np.float32(1.0) / np.float32(W - 1),
                     np.float32(1.0) / np.float32(D - 1)] * NCH, dtype=np.float32)
    mul12 = np.array([W * D, D, 1] * NCH, dtype=np.float32)
    cst[:, 0:12] = scl
    cst[:, 12:24] = kcol
    cst[:, 24:36] = mul12
    cst[:, 36] = np.nan
    return cst


def kernel(x: np.ndarray, coords: np.ndarray) -> np.ndarray:
    x = np.ascontiguousarray(x, dtype=np.float32)
    coords = np.ascontiguousarray(coords, dtype=np.float32)
    if "nc" not in _NC_CACHE:
        _NC_CACHE["nc"] = _build()
    nc = _NC_CACHE["nc"]
    cst = _consts()
    in_maps = []
    for i in range(NCORES):
        in_maps.append({
            "x": x,
            "coords": coords[i * NS:(i + 1) * NS],
            "cst": cst,
        })
    res = run_bass_kernel_spmd(nc, in_maps, list(range(NCORES))).results
    out = np.empty((B, N, C), dtype=np.float32)
    for i in range(NCORES):
        out[:, i * NS:(i + 1) * NS, :] = res[i]["out"]
    return out


# revision 5
# speedup vs baseline: 1.1900x; 1.1900x over previous
"""RBF-softmax grid pooling (CViT) on 8 TRN2 NeuronCores.

out[b,n,c] = sum_g softmax_g(-EPS*|coords_n - grid_g|^2) * x[b,g,c]

EPS=1e5 makes the softmax numerically supported on the 2x2x2 grid box around
each query: every non-box weight is < 1e-11 relative (below f32 resolution of
the sum), so the kernel gathers the 8 box corners per query and reproduces the
reference computation exactly on them, including XLA-CPU f32 exp flush-to-zero
semantics (exp(a)==0 iff a < -87.33654f) and the resulting 0/0 -> NaN rows.

Sharding: queries split across 8 cores (512 each). x is replicated, host-
transposed to (H, W, D, B, C) so one gather descriptor covers the (k0,k0+1) x
(b0,b1) block: 4 rows x 512B = 2KB per descriptor, 4 descriptors per query
(the i/j corner pairs), 2048 descriptors per core.
"""
import numpy as np
import concourse.bass as bass
import concourse.mybir as mybir
from concourse.bass_utils import run_bass_kernel_spmd
from contextlib import ExitStack

F32 = mybir.dt.float32
I32 = mybir.dt.int32
AF = mybir.ActivationFunctionType
OP = mybir.AluOpType

B, H, W, D, C = 2, 64, 64, 16, 128
N = 4096
NCORES = 8
NS = N // NCORES            # 512 queries per core
NCH = NS // 128             # 4 chunks of 128 queries
EPS = 100000.0
T_CUT = float(np.float32(-87.33654))    # last f32 input with XLA exp > 0
F_ZERO = float(np.float32(-87.33655))   # first f32 input with XLA exp == 0
# xt layout (H, W, D, B, C): row index of (i,j,k,b) = ((i*W+j)*D+k)*B+b
RMI = W * D * B             # 2048 rows per i step
RMJ = D * B                 # 32 rows per j step
RMK = B                     # 2 rows per k step

_NC_CACHE = {}


def _build():
    nc = bass.Bass()
    x_p = nc.declare_dram_parameter("xt", [H, W, D, B, C], F32, isOutput=False)
    co_p = nc.declare_dram_parameter("coords", [NS, 3], F32, isOutput=False)
    cs_p = nc.declare_dram_parameter("cst", [128, 37], F32, isOutput=False)
    out_p = nc.declare_dram_parameter("out", [B, NS, C], F32, isOutput=True)

    x_tab = x_p[:].rearrange("h w d b c -> (h w d b) c")

    with ExitStack() as ctx:
        def sb(name, shape, dt=F32):
            return ctx.enter_context(nc.sbuf_tensor(name, shape, dt))

        Cc = sb("Cc", [128, 12])        # coords: col = ch*3 + ax
        csb = sb("csb", [128, 37])      # scl(12) kcol(12) mulr(12) nan(1)
        Tt = sb("Tt", [128, 12])
        TM = sb("TM", [128, 12])
        I0i = sb("I0i", [128, 12], I32)
        I0f = sb("I0f", [128, 12])
        RBF = sb("RBF", [128, 12])
        RBS = sb("RBS", [128, 4])
        IDX = sb("IDX", [128, 4], I32)
        G0 = sb("G0", [128, 12])
        G1p = sb("G1p", [128, 12])
        G1 = sb("G1", [128, 12])
        DT = sb("DT", [128, 24])        # col = r*12 + ch*3 + ax
        QT = sb("QT", [128, 24])
        D2 = sb("D2", [128, 32])        # col = ch*8 + c, c = r*4+s*2+z
        Aa = sb("Aa", [128, 32])
        AM = sb("AM", [128, 32])
        MK = sb("MK", [128, 32])
        Ee = sb("Ee", [128, 32])
        EM = sb("EM", [128, 32])
        DEN = sb("DEN", [128, 4])
        RD = sb("RD", [128, 4])
        ZC = sb("ZC", [128, 4])
        IZ = sb("IZ", [128, 4])
        IZI = sb("IZI", [128, 4], I32)
        NB = sb("NB", [128, 4], I32)
        Ww = sb("Ww", [128, 32])
        # gathered tiles: per (ch, rs): [128, 512] = (z, b, C)
        F_t = [sb(f"F{g}", [128, 512]) for g in range(16)]
        # weighted tiles: per (ch, b, rs): [128, 256] = (z, C)
        P_t = [sb(f"P{k}", [128, 256]) for k in range(32)]
        SA = [[sb(f"SA{p}_{j}", [128, 256]) for j in range(2)] for p in range(2)]
        SS = [sb(f"SS{p}", [128, 256]) for p in range(2)]
        ACC = [sb(f"ACC{bi}", [128, 128]) for bi in range(8)]

        sd = ctx.enter_context(nc.semaphore("sd"))
        sv = ctx.enter_context(nc.semaphore("sv"))
        sa = ctx.enter_context(nc.semaphore("sa"))
        sg = ctx.enter_context(nc.semaphore("sg"))
        ss = ctx.enter_context(nc.semaphore("ss"))
        so = ctx.enter_context(nc.semaphore("so"))
        block = ctx.enter_context(nc.Block())

        scl = csb[:, 0:12]
        kcol = csb[:, 12:24]
        mulr = csb[:, 24:36]
        nanb = csb[:, 36:37]

        @block.sync
        def _(sync):
            sync.dma_start(
                Cc[:].rearrange("p (ch ax) -> p ch ax", ch=NCH, ax=3),
                co_p[:].rearrange("(ch p) ax -> p ch ax", p=128),
            ).then_inc(sd, 16)
            sync.dma_start(csb[:], cs_p[:]).then_inc(sd, 16)
            for bi in range(8):
                ch, b = bi // 2, bi % 2
                sync.wait_ge(ss, bi + 1)
                sync.dma_start(out_p[b, ch * 128:(ch + 1) * 128, :],
                               ACC[bi][:]).then_inc(so, 16)
            sync.wait_ge(so, 128)

        @block.gpsimd
        def _(gpsimd):
            gpsimd.wait_ge(sv, 1)       # IDX ready
            for ch in range(NCH):
                for rs in range(4):
                    r, s = rs >> 1, rs & 1
                    eo = (r * RMI + s * RMJ) * C
                    gpsimd.indirect_dma_start(
                        out=F_t[ch * 4 + rs][:],
                        out_offset=None,
                        in_=x_tab,
                        in_offset=bass.IndirectOffsetOnAxis(
                            ap=IDX[:, ch:ch + 1], axis=0),
                        element_offset=eo,
                    ).then_inc(sg, 16)

        # ACT: exp, then for each (ch, b): rs=2,3 slice-mults (z=0,1 each)
        @block.scalar
        def _(scalar):
            scalar.wait_ge(sv, 2)       # AM ready
            scalar.activation(Ee[:], AM[:], AF.Exp).then_inc(sa, 1)
            scalar.wait_ge(sv, 3)       # W ready
            for ch in range(NCH):
                for b in range(B):
                    for rs in (2, 3):
                        scalar.wait_ge(sg, 16 * (ch * 4 + rs + 1))
                        k = (ch * 2 + b) * 4 + rs
                        for z in range(2):
                            # F[ch*4+rs][:, (z*2+b)*128 : +128] * W[:, ch*8+rs*2+z]
                            scalar.activation(
                                P_t[k][:, z * 128:(z + 1) * 128],
                                F_t[ch * 4 + rs][:, (z * 2 + b) * 128:(z * 2 + b + 1) * 128],
                                AF.Copy,
                                scale=Ww[:, ch * 8 + rs * 2 + z: ch * 8 + rs * 2 + z + 1],
                            ).then_inc(sa, 1)

        @block.vector
        def _(v):
            v.wait_ge(sd, 32)
            # ---- index chain first (critical path to gathers) ----
            v.tensor_tensor(Tt[:], Cc[:], scl, OP.mult)
            v.drain()
            v.tensor_scalar_sub(TM[:], Tt[:], 0.5)
            v.drain()
            v.tensor_copy(I0i[:], TM[:])            # rint(t-0.5) == floor(t)
            v.drain()
            v.tensor_copy(I0f[:], I0i[:])
            v.drain()
            v.tensor_tensor(RBF[:], I0f[:], mulr, OP.mult)
            v.drain()
            v.tensor_reduce(RBS[:], RBF[:].rearrange("p (ch ax) -> p ch ax", ch=NCH),
                            axis=mybir.AxisListType.X, op=OP.add)
            v.drain()
            v.tensor_copy(IDX[:], RBS[:]).then_inc(sv, 1)   # gathers may start
            # ---- weights (overlaps gathers) ----
            v.tensor_tensor(G0[:], I0f[:], kcol, OP.mult)
            v.tensor_scalar_add(G1p[:], I0f[:], 1.0)
            v.drain()
            v.tensor_tensor(G1[:], G1p[:], kcol, OP.mult)
            v.tensor_tensor(DT[:, 0:12], Cc[:], G0[:], OP.subtract)
            v.drain()
            v.tensor_tensor(DT[:, 12:24], Cc[:], G1[:], OP.subtract)
            v.drain()
            v.tensor_tensor(QT[:], DT[:], DT[:], OP.mult)
            v.drain()
            QTv = QT[:].rearrange("p (r ch ax) -> p r ch ax", r=2, ch=NCH, ax=3)
            D2v = D2[:].rearrange("p (ch c) -> p c ch", ch=NCH, c=8)
            for c in range(8):
                r, s = (c >> 2) & 1, (c >> 1) & 1
                v.tensor_tensor(D2v[:, c], QTv[:, r, :, 0], QTv[:, s, :, 1], OP.add)
            v.drain()
            for c in range(8):
                z = c & 1
                v.tensor_tensor(D2v[:, c], D2v[:, c], QTv[:, z, :, 2], OP.add)
            v.drain()
            v.tensor_scalar_mul(Aa[:], D2[:], -EPS)
            v.drain()
            v.tensor_scalar_max(AM[:], Aa[:], T_CUT)
            v.tensor_scalar(MK[:], Aa[:], F_ZERO, 1e9, OP.subtract, OP.mult)
            v.drain()
            v.tensor_scalar(MK[:], MK[:], 0.0, 1.0, OP.max, OP.min).then_inc(sv, 1)
            v.drain()
            v.wait_ge(sa, 1)            # E ready
            v.tensor_tensor(EM[:], Ee[:], MK[:], OP.mult)
            v.drain()
            v.tensor_reduce(DEN[:], EM[:].rearrange("p (ch c) -> p ch c", ch=NCH),
                            axis=mybir.AxisListType.X, op=OP.add)
            v.drain()
            v.reciprocal(RD[:], DEN[:])
            v.tensor_scalar(ZC[:], DEN[:], 3.4e38, 1.0, OP.mult, OP.min)
            v.drain()
            v.tensor_scalar_sub(IZ[:], ZC[:], 1.0)      # -1 dead, 0 alive
            v.drain()
            v.tensor_copy(IZI[:], IZ[:])
            v.drain()
            v.tensor_tensor(NB[:], IZI[:],
                            nanb.bitcast(I32).to_broadcast([128, 4]),
                            OP.bitwise_and)
            for ch in range(NCH):
                v.tensor_scalar_mul(Ww[:, ch * 8:(ch + 1) * 8],
                                    EM[:, ch * 8:(ch + 1) * 8],
                                    RD[:, ch:ch + 1])
            v.drain()
            last = None
            for ch in range(NCH):
                last = v.tensor_tensor(
                    Ww[:, ch * 8:(ch + 1) * 8].bitcast(I32),
                    Ww[:, ch * 8:(ch + 1) * 8].bitcast(I32),
                    NB[:, ch:ch + 1].to_broadcast([128, 8]),
                    OP.bitwise_or)
            last.then_inc(sv, 1)        # W ready
            v.drain()
            # ---- per-(ch,b) weighted reduce ----
            # DVE handles rs=0,1 wide mults; ACT handles rs=2,3.
            # P tile layout per (ch,b,rs): [128, (z,C)] = [128, 256].
            for bi in range(8):
                ch, b = bi // 2, bi % 2
                par = bi % 2
                for rs in (0, 1):
                    v.wait_ge(sg, 16 * (ch * 4 + rs + 1))
                    k = bi * 4 + rs
                    # in0: F[ch*4+rs] view [(z: step 2*128, 2), (C: 128)] offset b*128
                    fv = F_t[ch * 4 + rs][:].rearrange(
                        "p (z b2 c) -> p z b2 c", z=2, b2=B)[:, :, b, :]
                    wv = Ww[:, ch * 8 + rs * 2: ch * 8 + rs * 2 + 2].to_broadcast(
                        [128, 2, 128])
                    v.tensor_tensor(
                        P_t[k][:].rearrange("p (z c) -> p z c", z=2),
                        fv, wv, OP.mult)
                v.drain()
                v.wait_ge(sa, 1 + 4 * (bi + 1))      # ACT mults for this block
                v.tensor_tensor(SA[par][0][:], P_t[bi * 4 + 0][:],
                                P_t[bi * 4 + 1][:], OP.add)
                v.tensor_tensor(SA[par][1][:], P_t[bi * 4 + 2][:],
                                P_t[bi * 4 + 3][:], OP.add)
                v.drain()
                v.tensor_tensor(SS[par][:], SA[par][0][:], SA[par][1][:], OP.add)
                v.drain()
                v.tensor_tensor(ACC[bi][:], SS[par][:, 0:128], SS[par][:, 128:256],
                                OP.add).then_inc(ss, 1)
    return nc


def _consts():
    cst = np.zeros((128, 37), dtype=np.float32)
    scl = np.array([H - 1, W - 1, D - 1] * NCH, dtype=np.float32)
    kcol = np.array([np.float32(1.0) / np.float32(H - 1),
                     np.float32(1.0) / np.float32(W - 1),
                     np.float32(1.0) / np.float32(D - 1)] * NCH, dtype=np.float32)
    mulr = np.array([RMI, RMJ, RMK] * NCH, dtype=np.float32)
    cst[:, 0:12] = scl
    cst[:, 12:24] = kcol
    cst[:, 24:36] = mulr
    cst[:, 36] = np.nan
    return cst


def make_in_maps(x: np.ndarray, coords: np.ndarray):
    x = np.ascontiguousarray(x, dtype=np.float32)
    coords = np.ascontiguousarray(coords, dtype=np.float32)
    xt = np.ascontiguousarray(x.transpose(1, 2, 3, 0, 4))   # (H, W, D, B, C)
    cst = _consts()
    return [{
        "xt": xt,
        "coords": coords[i * NS:(i + 1) * NS],
        "cst": cst,
    } for i in range(NCORES)]


def kernel(x: np.ndarray, coords: np.ndarray) -> np.ndarray:
    if "nc" not in _NC_CACHE:
        _NC_CACHE["nc"] = _build()
    nc = _NC_CACHE["nc"]
    in_maps = make_in_maps(x, coords)
    res = run_bass_kernel_spmd(nc, in_maps, list(range(NCORES))).results
    out = np.empty((B, N, C), dtype=np.float32)
    for i in range(NCORES):
        out[:, i * NS:(i + 1) * NS, :] = res[i]["out"]
    return out


# revision 9
# speedup vs baseline: 1.4048x; 1.1805x over previous
"""RBF-softmax grid pooling (CViT) on 8 TRN2 NeuronCores.

out[b,n,c] = sum_g softmax_g(-EPS*|coords_n - grid_g|^2) * x[b,g,c]

EPS=1e5 makes the softmax numerically supported on the 2x2x2 grid box around
each query: every non-box weight is < 1e-11 relative (below f32 resolution of
the sum), so the kernel gathers the 8 box corners per query and reproduces the
reference computation exactly on them, including XLA-CPU f32 exp flush-to-zero
semantics (exp(a)==0 iff a < -87.33654f) and the resulting 0/0 -> NaN rows.

Sharding: queries split across 8 cores (512 each). x is replicated, host-
transposed to (H, W, D, B, C) so one gather descriptor covers the (k0,k0+1) x
(b0,b1) block: 4 rows x 512B = 2KB per descriptor, 4 descriptors per query
(the i/j corner pairs), 2048 descriptors per core.
"""
import numpy as np
import concourse.bass as bass
import concourse.mybir as mybir
from concourse.bass_utils import run_bass_kernel_spmd
from contextlib import ExitStack

F32 = mybir.dt.float32
I32 = mybir.dt.int32
AF = mybir.ActivationFunctionType
OP = mybir.AluOpType

B, H, W, D, C = 2, 64, 64, 16, 128
N = 4096
NCORES = 8
NS = N // NCORES            # 512 queries per core
NCH = NS // 128             # 4 chunks of 128 queries
EPS = 100000.0
T_CUT = float(np.float32(-87.33654))    # last f32 input with XLA exp > 0
F_ZERO = float(np.float32(-87.33655))   # first f32 input with XLA exp == 0
# xt layout (H, W, D, B, C): row index of (i,j,k,b) = ((i*W+j)*D+k)*B+b
RMI = W * D * B             # 2048 rows per i step
RMJ = D * B                 # 32 rows per j step
RMK = B                     # 2 rows per k step

_NC_CACHE = {}


def _build():
    nc = bass.Bass()
    x_p = nc.declare_dram_parameter("xt", [H, W, D, B, C], F32, isOutput=False)
    mt_p = nc.declare_dram_parameter("meta", [128, 49], F32, isOutput=False)
    out_p = nc.declare_dram_parameter("out", [B, NS, C], F32, isOutput=True)

    x_tab = x_p[:].rearrange("h w d b c -> (h w d b) c")

    with ExitStack() as ctx:
        def sb(name, shape, dt=F32):
            return ctx.enter_context(nc.sbuf_tensor(name, shape, dt))

        MT = sb("MT", [128, 49])
        Tt = sb("Tt", [128, 12])
        TM = sb("TM", [128, 12])
        I0i = sb("I0i", [128, 12], I32)
        I0f = sb("I0f", [128, 12])
        RBFi = sb("RBFi", [128, 12], I32)
        IDX = sb("IDX", [128, 4], I32)
        G0 = sb("G0", [128, 12])
        G1p = sb("G1p", [128, 12])
        G1 = sb("G1", [128, 12])
        DT = sb("DT", [128, 24])        # col = r*12 + ch*3 + ax
        QT = sb("QT", [128, 24])
        D2 = sb("D2", [128, 32])        # col = ch*8 + c, c = r*4+s*2+z
        D2a = sb("D2a", [128, 16])      # col = ch*4 + r*2 + s
        Aa = sb("Aa", [128, 32])
        AM = sb("AM", [128, 32])
        MK = sb("MK", [128, 32])
        Ee = sb("Ee", [128, 32])
        EM = sb("EM", [128, 32])
        DEN = sb("DEN", [128, 4])
        RD = sb("RD", [128, 4])
        ZC = sb("ZC", [128, 4])
        IZ = sb("IZ", [128, 4])
        IZI = sb("IZI", [128, 4], I32)
        NB = sb("NB", [128, 4], I32)
        Ww = sb("Ww", [128, 32])
        # gathered tiles: per (ch, rs): [128, 512] = (z, b, C)
        F_t = [sb(f"F{g}", [128, 512]) for g in range(16)]
        # weighted block tiles: per (ch,b): [128, 1024] = (rs, z, C)
        PB = [sb(f"PB{bi}", [128, 1024]) for bi in range(8)]
        ACC = [sb(f"ACC{bi}", [128, 128]) for bi in range(8)]

        sd = ctx.enter_context(nc.semaphore("sd"))
        sv = ctx.enter_context(nc.semaphore("sv"))
        sa = ctx.enter_context(nc.semaphore("sa"))
        sg = ctx.enter_context(nc.semaphore("sg"))
        ss = ctx.enter_context(nc.semaphore("ss"))
        so = ctx.enter_context(nc.semaphore("so"))
        block = ctx.enter_context(nc.Block())

        Cc = MT[:, 0:12]
        scl = MT[:, 12:24]
        kcol = MT[:, 24:36]
        mulri = MT[:, 36:48]            # int32 bits stored in f32 tensor
        nanb = MT[:, 48:49]

        @block.sync
        def _(sync):
            sync.dma_start(MT[:], mt_p[:]).then_inc(sd, 16)
            for bi in range(8):
                ch, b = bi // 2, bi % 2
                sync.wait_ge(ss, bi + 1)
                sync.dma_start(out_p[b, ch * 128:(ch + 1) * 128, :],
                               ACC[bi][:]).then_inc(so, 16)
            sync.wait_ge(so, 128)

        @block.gpsimd
        def _(gpsimd):
            gpsimd.wait_ge(sv, 1)       # IDX ready
            for ch in range(NCH):
                for rs in range(4):
                    r, s = rs >> 1, rs & 1
                    eo = (r * RMI + s * RMJ) * C
                    gpsimd.indirect_dma_start(
                        out=F_t[ch * 4 + rs][:],
                        out_offset=None,
                        in_=x_tab,
                        in_offset=bass.IndirectOffsetOnAxis(
                            ap=IDX[:, ch:ch + 1], axis=0),
                        element_offset=eo,
                    ).then_inc(sg, 16)

        # ACT: exp, then per (ch,b): rs=2,3 slice-mults into PB
        @block.scalar
        def _(scalar):
            scalar.wait_ge(sv, 2)       # AM ready
            scalar.activation(Ee[:], AM[:], AF.Exp).then_inc(sa, 1)
            scalar.wait_ge(sv, 3)       # W ready
            for ch in range(NCH):
                for b in range(B):
                    bi = ch * 2 + b
                    for rs in (2, 3):
                        scalar.wait_ge(sg, 16 * (ch * 4 + rs + 1))
                        for z in range(2):
                            col = rs * 2 + z
                            scalar.activation(
                                PB[bi][:, col * 128:(col + 1) * 128],
                                F_t[ch * 4 + rs][:, (z * 2 + b) * 128:(z * 2 + b + 1) * 128],
                                AF.Copy,
                                scale=Ww[:, ch * 8 + col: ch * 8 + col + 1],
                            ).then_inc(sa, 1)

        @block.vector
        def _(v):
            v.wait_ge(sd, 16)
            # ---- index chain first (critical path to gathers) ----
            v.tensor_tensor(Tt[:], Cc, scl, OP.mult)
            v.drain()
            v.tensor_scalar_sub(TM[:], Tt[:], 0.5)
            v.drain()
            v.tensor_copy(I0i[:], TM[:])            # rint(t-0.5) == floor(t)
            v.drain()
            v.tensor_tensor(RBFi[:], I0i[:], mulri.bitcast(I32), OP.mult)
            v.drain()
            with nc.allow_low_precision(reason="int32 index sums are exact"):
                v.tensor_reduce(IDX[:],
                                RBFi[:].rearrange("p (ch ax) -> p ch ax", ch=NCH),
                                axis=mybir.AxisListType.X,
                                op=OP.add).then_inc(sv, 1)
            v.drain()
            # ---- weights (overlaps gathers) ----
            v.tensor_copy(I0f[:], I0i[:])
            v.drain()
            v.tensor_tensor(G0[:], I0f[:], kcol, OP.mult)
            v.tensor_scalar_add(G1p[:], I0f[:], 1.0)
            v.drain()
            v.tensor_tensor(G1[:], G1p[:], kcol, OP.mult)
            v.tensor_tensor(DT[:, 0:12], Cc, G0[:], OP.subtract)
            v.drain()
            v.tensor_tensor(DT[:, 12:24], Cc, G1[:], OP.subtract)
            v.drain()
            v.tensor_tensor(QT[:], DT[:], DT[:], OP.mult)
            v.drain()
            # D2[p, ch, r, s, z] = QX[r] + QY[s] + QZ[z] via 3D broadcast APs
            qb = QT[:]
            pstep = qb.ap[0][0]
            QX = bass.AP(qb.tensor, qb.offset + 0,
                         [[pstep, 128], [3, NCH], [12, 2], [0, 2]])
            QY = bass.AP(qb.tensor, qb.offset + 1,
                         [[pstep, 128], [3, NCH], [0, 2], [12, 2]])
            D2a_v = D2a[:].rearrange("p (ch r s) -> p ch r s", ch=NCH, r=2, s=2)
            v.tensor_tensor(D2a_v, QX, QY, OP.add)
            v.drain()
            da = D2a[:]
            d2b = D2[:]
            QZ = bass.AP(qb.tensor, qb.offset + 2,
                         [[pstep, 128], [3, NCH], [0, 4], [12, 2]])
            in0 = bass.AP(da.tensor, da.offset,
                          [[da.ap[0][0], 128], [4, NCH], [1, 4], [0, 2]])
            outv = bass.AP(d2b.tensor, d2b.offset,
                           [[d2b.ap[0][0], 128], [8, NCH], [2, 4], [1, 2]])
            v.tensor_tensor(outv, in0, QZ, OP.add)
            v.drain()
            v.tensor_scalar_mul(Aa[:], D2[:], -EPS)
            v.drain()
            v.tensor_scalar_max(AM[:], Aa[:], T_CUT)
            v.tensor_scalar(MK[:], Aa[:], F_ZERO, 1e9, OP.subtract, OP.mult)
            v.drain()
            v.tensor_scalar(MK[:], MK[:], 0.0, 1.0, OP.max, OP.min).then_inc(sv, 1)
            v.drain()
            v.wait_ge(sa, 1)            # E ready
            v.tensor_tensor(EM[:], Ee[:], MK[:], OP.mult)
            v.drain()
            v.tensor_reduce(DEN[:], EM[:].rearrange("p (ch c) -> p ch c", ch=NCH),
                            axis=mybir.AxisListType.X, op=OP.add)
            v.drain()
            v.reciprocal(RD[:], DEN[:])
            v.tensor_scalar(ZC[:], DEN[:], 3.4e38, 1.0, OP.mult, OP.min)
            v.drain()
            v.tensor_scalar_sub(IZ[:], ZC[:], 1.0)      # -1 dead, 0 alive
            v.drain()
            v.tensor_copy(IZI[:], IZ[:])
            v.drain()
            v.tensor_tensor(NB[:], IZI[:],
                            nanb.bitcast(I32).to_broadcast([128, 4]),
                            OP.bitwise_and)
            for ch in range(NCH):
                v.tensor_scalar_mul(Ww[:, ch * 8:(ch + 1) * 8],
                                    EM[:, ch * 8:(ch + 1) * 8],
                                    RD[:, ch:ch + 1])
            v.drain()
            last = None
            for ch in range(NCH):
                last = v.tensor_tensor(
                    Ww[:, ch * 8:(ch + 1) * 8].bitcast(I32),
                    Ww[:, ch * 8:(ch + 1) * 8].bitcast(I32),
                    NB[:, ch:ch + 1].to_broadcast([128, 8]),
                    OP.bitwise_or)
            last.then_inc(sv, 1)        # W ready
            v.drain()
            # ---- per-(ch,b) weighted reduce: DVE rs=0,1 mults + one reduce ----
            for bi in range(8):
                ch, b = bi // 2, bi % 2
                for rs in (0, 1):
                    v.wait_ge(sg, 16 * (ch * 4 + rs + 1))
                    fv = F_t[ch * 4 + rs][:].rearrange(
                        "p (z b2 c) -> p z b2 c", z=2, b2=B)[:, :, b, :]
                    wv = Ww[:, ch * 8 + rs * 2: ch * 8 + rs * 2 + 2].to_broadcast(
                        [128, 2, 128])
                    v.tensor_tensor(
                        PB[bi][:, rs * 256:(rs + 1) * 256].rearrange(
                            "p (z c) -> p z c", z=2),
                        fv, wv, OP.mult)
                v.drain()
                v.wait_ge(sa, 1 + 4 * (bi + 1))      # ACT mults for this block
                pb = PB[bi][:]
                rin = bass.AP(pb.tensor, pb.offset,
                              [[pb.ap[0][0], 128], [1, 128], [128, 8]])
                v.tensor_reduce(ACC[bi][:], rin,
                                axis=mybir.AxisListType.X,
                                op=OP.add).then_inc(ss, 1)
    return nc


def _consts():
    cst = np.zeros((128, 37), dtype=np.float32)
    cst[:, 0:12] = np.array([H - 1, W - 1, D - 1] * NCH, dtype=np.float32)
    cst[:, 12:24] = np.array([np.float32(1.0) / np.float32(H - 1),
                              np.float32(1.0) / np.float32(W - 1),
                              np.float32(1.0) / np.float32(D - 1)] * NCH,
                             dtype=np.float32)
    cst[:, 24:36] = np.array([RMI, RMJ, RMK] * NCH,
                             dtype=np.int32).view(np.float32)
    cst[:, 36] = np.nan
    return cst


def make_in_maps(x: np.ndarray, coords: np.ndarray):
    x = np.ascontiguousarray(x, dtype=np.float32)
    coords = np.ascontiguousarray(coords, dtype=np.float32)
    xt = np.ascontiguousarray(x.transpose(1, 2, 3, 0, 4))   # (H, W, D, B, C)
    cst = _consts()
    maps = []
    for i in range(NCORES):
        cs = coords[i * NS:(i + 1) * NS]
        cl = cs.reshape(NCH, 128, 3).transpose(1, 0, 2).reshape(128, 12)
        meta = np.concatenate([cl, cst], axis=1).astype(np.float32)
        maps.append({"xt": xt, "meta": meta})
    return maps


def kernel(x: np.ndarray, coords: np.ndarray) -> np.ndarray:
    if "nc" not in _NC_CACHE:
        _NC_CACHE["nc"] = _build()
    nc = _NC_CACHE["nc"]
    in_maps = make_in_maps(x, coords)
    res = run_bass_kernel_spmd(nc, in_maps, list(range(NCORES))).results
    out = np.empty((B, N, C), dtype=np.float32)
    for i in range(NCORES):
        out[:, i * NS:(i + 1) * NS, :] = res[i]["out"]
    return out
